# revision 1
# baseline (speedup 1.0000x reference)
"""Trainium2 Bass kernel for nn_Attention_4_lora (B=8, T=1024, C=1024, R=64).

Strategy: data-parallel over the batch dim (1 batch per NeuronCore, 8 cores).
All activations live in transposed [channel, token] layout so that every
matmul contraction runs over the SBUF partition axis. BatchNorm statistics
are reduced across cores with one small (24 KB) AllReduce. All heavy matmuls
run in float32r (TF32-like, full PE throughput at N>=256, ~1e-4 rounding).

Per-core pipeline:
  P1  merge Wm_attn^T = W_attn^T + reshape(A@B)^T on device, in d-quarters
      (the torch .view row-major reshape interleaves the LoRA delta with
      stride 3 in the transposed layout; handled with strided SBUF views)
  P2  xa^T[d, t] = Wm^T-slab.T @ x^T  for q,k channels + bn_stats per tile
  P3  v[t, c] (natural layout, needed as AV stationary) + ones-matmul stats
  P4  AllReduce (sum over cores of per-channel mean/E[x^2]) -> normalize
  P5  scores^T[s, t] = k^T-slab.T @ q^T, exp((q.k)/32) on ScalarE,
      causal mask via affine_select (exact zeros), row-sums via ones-matmul
  P6  y^T[c, t] = v-slab.T @ att_exp^T, fused 1/r normalize on PSUM drain
  P7  y1^T = Wp^T-slab.T @ y^T ; y2^T = Wmp^T-slab.T @ y1^T -> out [C, T]

kernel() takes the full unsharded inputs, shards/uploads, runs SPMD on
cores 0-7, gathers, and transposes back to [B, T, C].
"""

import numpy as np

import concourse.bass as bass
import concourse.mybir as mybir
import concourse.tile as tile
from concourse import bacc
from concourse.bass_utils import run_bass_kernel_spmd

NCORES = 8
C = 1024
R = 64
D3 = 3 * C
EPS = 1e-5
F32 = mybir.dt.float32
F32R = mybir.dt.float32r
AX = mybir.AxisListType
OP = mybir.AluOpType
ACTF = mybir.ActivationFunctionType


def _erange(f, d0, d1):
    """e-range such that d = 3e + f lies in [d0, d1)."""
    el = -((-(d0 - f)) // 3)
    eh = -((-(d1 - f)) // 3)
    return el, eh


def build(T=1024, single_core=False, no_collective=False, reps=1):
    NT = T // 128          # 128-token tiles
    TQ = T // 512          # 512-token chunks
    assert T % 512 == 0

    nc = bacc.Bacc(None, target_bir_lowering=False,
                   num_devices=(1 if single_core else NCORES))

    prm = {}
    prm["xT"] = nc.declare_dram_parameter("xT", [C, T], F32R, isOutput=False)
    prm["wT"] = nc.declare_dram_parameter("wT", [C, D3], F32R, isOutput=False)
    prm["wpT"] = nc.declare_dram_parameter("wpT", [C, C], F32R, isOutput=False)
    prm["laT"] = nc.declare_dram_parameter("laT", [R, C], F32R, isOutput=False)
    prm["lbB"] = nc.declare_dram_parameter("lbB", [R, D3], F32R, isOutput=False)
    prm["lpaT"] = nc.declare_dram_parameter("lpaT", [R, C], F32R, isOutput=False)
    prm["lpbB"] = nc.declare_dram_parameter("lpbB", [R, C], F32R, isOutput=False)
    prm["gam"] = nc.declare_dram_parameter("gam", [D3], F32, isOutput=False)
    prm["bet"] = nc.declare_dram_parameter("bet", [D3], F32, isOutput=False)
    prm["out"] = nc.declare_dram_parameter("out", [C, T], F32, isOutput=True)

    with tile.TileContext(nc) as tc:
        for rep in range(reps):
            _emit(nc, tc, prm, T, rep, single_core, no_collective)

    nc.compile()
    return nc


def _emit(nc, tc, prm, T, rep, single_core, no_collective):
    NT = T // 128
    TQ = T // 512
    xT, wT, wpT, laT, lbB = prm["xT"], prm["wT"], prm["wpT"], prm["laT"], prm["lbB"]
    lpaT, lpbB, gam, bet, out = prm["lpaT"], prm["lpbB"], prm["gam"], prm["bet"], prm["out"]

    stats_in = nc.dram_tensor(f"stats_in_{rep}", [4096], F32)
    stats_out = nc.dram_tensor(f"stats_out_{rep}", [4096], F32)
    vstats_in = nc.dram_tensor(f"vstats_in_{rep}", [2 * C], F32)
    vstats_out = nc.dram_tensor(f"vstats_out_{rep}", [2 * C], F32)
    rb_dram = nc.dram_tensor(f"rb_{rep}", [T], F32)

    def bcast_dram(param, offset, n):
        return bass.AP(tensor=param[:].tensor, offset=offset, ap=[[0, 128], [1, n]])

    with (
        tc.tile_pool(name=f"misc{rep}", bufs=1) as misc,
        tc.tile_pool(name=f"outst{rep}", bufs=2) as outst,
        tc.tile_pool(name=f"vpool{rep}", bufs=1) as vpool,
        tc.tile_pool(name=f"attp{rep}", bufs=1) as attp,
        tc.tile_pool(name=f"psA{rep}", bufs=4, space="PSUM") as psA,
    ):
        # ---------------- constants / small loads ----------------
        ones_f = misc.tile([128, 1], F32)
        nc.vector.memset(ones_f[:, :], 1.0)
        ones_r = misc.tile([128, 1], F32R)
        nc.vector.tensor_copy(out=ones_r[:, :], in_=ones_f[:, :])
        eps_t = misc.tile([128, 1], F32)
        nc.vector.memset(eps_t[:, :], EPS)

        gqk = misc.tile([128, 16], F32)
        nc.sync.dma_start(out=gqk[:, :], in_=gam[0:2048].rearrange("(i p) -> p i", p=128))
        bqk = misc.tile([128, 16], F32)
        nc.sync.dma_start(out=bqk[:, :], in_=bet[0:2048].rearrange("(i p) -> p i", p=128))

        qk_mv = misc.tile([128, 16, 2], F32)
        m16 = misc.tile([128, 16], F32)
        qa = misc.tile([128, 16], F32)
        qb = misc.tile([128, 16], F32)
        r_bc = misc.tile([128, T], F32)

        xa = [None] * 16
        vnat = [None] * NT

        with tc.tile_pool(name=f"xapool{rep}", bufs=1) as xapool:
            with tc.tile_pool(name=f"lorap{rep}", bufs=1) as lorap:
                la_sb = lorap.tile([R, C], F32R)
                nc.sync.dma_start(out=la_sb[:, :], in_=laT[:, :])
                lb_sb = lorap.tile([R, D3], F32R)
                for _c in range(3):
                    nc.sync.dma_start(out=lb_sb[:, 1024 * _c:1024 * (_c + 1)],
                                      in_=lbB[:, 1024 * _c:1024 * (_c + 1)])

                with tc.tile_pool(name=f"xtpool{rep}", bufs=1) as xtpool:
                    with tc.tile_pool(name=f"wb{rep}", bufs=1) as wbp:
                        # ---------------- P1+P2: q,k weight quarters + xa pass
                        # ---------------- then P3: v quarters + natural-v pass
                        bnstat = None

                        def merge_quarter(d0):
                            """Merged Wm^T[:, d0:d0+512] as 8 c-tiles [128, 516]."""
                            wq = []
                            for ct in range(8):
                                w_t = wbp.tile([128, 516], F32R, tag=f"wb{ct}", bufs=1,
                                               name=f"wq{d0}_{ct}")
                                nc.sync.dma_start(
                                    out=w_t[:, 0:512],
                                    in_=wT[128 * ct:128 * (ct + 1), d0:d0 + 512])
                                view3 = w_t[:, :].rearrange("p (u three) -> p u three", three=3)
                                for f in range(3):
                                    el, eh = _erange(f, d0, d0 + 512)
                                    cnt = eh - el
                                    c0 = 3 * el + f - d0
                                    # f32r matmul needs an even moving free dim
                                    cnt_mm = cnt + (cnt % 2)
                                    es, off = el, 0
                                    if es + cnt_mm > C:
                                        es, off = el - 1, 1
                                    ps = psA.tile([128, 512], F32, tag="mm", name=f"dps{d0}_{ct}_{f}")
                                    nc.tensor.matmul(
                                        ps[:, 0:cnt_mm],
                                        lb_sb[:, 1024 * f + 128 * ct:1024 * f + 128 * (ct + 1)],
                                        la_sb[:, es:es + cnt_mm],
                                        start=True, stop=True)
                                    nc.vector.tensor_tensor(
                                        out=view3[:, 0:cnt, c0],
                                        in0=view3[:, 0:cnt, c0],
                                        in1=ps[:, off:off + cnt], op=OP.add)
                                wq.append(w_t)
                            return wq

                        wq0 = merge_quarter(0)
                        xt = []
                        for k in range(8):
                            x_t = xtpool.tile([128, T], F32R, tag=f"xt{k}", name=f"xt{k}")
                            nc.sync.dma_start(out=x_t[:, :], in_=xT[128 * k:128 * (k + 1), :])
                            xt.append(x_t)

                        for Q in range(4):           # q,k channels: d in [512Q, 512Q+512)
                            wq = wq0 if Q == 0 else merge_quarter(512 * Q)
                            for il in range(4):
                                g = 4 * Q + il
                                xa_g = xapool.tile([128, T], F32R, tag=f"xa{g}",
                                                   name=f"xa{g}")
                                for tch in range(TQ):
                                    ps = psA.tile([128, 512], F32, tag="mm", name=f"xaps{g}_{tch}")
                                    for k in range(8):
                                        nc.tensor.matmul(
                                            ps[:, :],
                                            wq[k][:, 128 * il:128 * (il + 1)],
                                            xt[k][:, 512 * tch:512 * (tch + 1)],
                                            start=(k == 0), stop=(k == 7))
                                    nc.scalar.copy(out=xa_g[:, 512 * tch:512 * (tch + 1)],
                                                   in_=ps[:, :])
                                bnstat = misc.tile([128, TQ, 6], F32, tag="bnstat",
                                                   bufs=2, name=f"bnstat{g}")
                                for j in range(TQ):
                                    nc.vector.bn_stats(out=bnstat[:, j, :],
                                                       in_=xa_g[:, 512 * j:512 * (j + 1)])
                                nc.vector.bn_aggr(out=qk_mv[:, g, :], in_=bnstat[:, :, :])
                                xa[g] = xa_g

                        # qk stats -> (mean, E[x^2]) packed, DMA to stats_in
                        nc.vector.tensor_tensor(out=m16[:, :], in0=qk_mv[:, :, 0],
                                                in1=qk_mv[:, :, 0], op=OP.mult)
                        nc.vector.tensor_tensor(out=qk_mv[:, :, 1], in0=qk_mv[:, :, 1],
                                                in1=m16[:, :], op=OP.add)
                        nc.sync.dma_start(
                            out=stats_in[0:4096].rearrange("(p i s) -> p i s", p=128, s=2),
                            in_=qk_mv[:, :, :])
                        if single_core or no_collective:
                            nc.sync.dma_start(out=stats_out[:], in_=stats_in[:])
                        else:
                            nc.gpsimd.collective_compute(
                                "AllReduce", OP.add,
                                replica_groups=[list(range(NCORES))],
                                ins=[stats_in[:]], outs=[stats_out[:]])
                        ar_qk = misc.tile([128, 16, 2], F32)
                        nc.sync.dma_start(
                            out=ar_qk[:, :, :],
                            in_=stats_out[0:4096].rearrange("(p i s) -> p i s", p=128, s=2))
                        # q,k: a = gamma*rstd, b = beta - mean*a (runs during P3)
                        nc.vector.tensor_scalar(out=ar_qk[:, :, 0], in0=ar_qk[:, :, 0],
                                                scalar1=1.0 / NCORES, scalar2=None, op0=OP.mult)
                        nc.vector.tensor_scalar(out=ar_qk[:, :, 1], in0=ar_qk[:, :, 1],
                                                scalar1=1.0 / NCORES, scalar2=None, op0=OP.mult)
                        nc.vector.tensor_tensor(out=m16[:, :], in0=ar_qk[:, :, 0],
                                                in1=ar_qk[:, :, 0], op=OP.mult)
                        nc.vector.tensor_tensor(out=m16[:, :], in0=ar_qk[:, :, 1],
                                                in1=m16[:, :], op=OP.subtract)
                        nc.scalar.activation(out=m16[:, :], in_=m16[:, :], func=ACTF.Sqrt,
                                             bias=eps_t[:, 0:1])
                        nc.vector.reciprocal(out=m16[:, :], in_=m16[:, :])
                        nc.vector.tensor_tensor(out=qa[:, :], in0=m16[:, :], in1=gqk[:, :],
                                                op=OP.mult)
                        nc.vector.tensor_tensor(out=qb[:, :], in0=ar_qk[:, :, 0], in1=qa[:, :],
                                                op=OP.mult)
                        nc.vector.tensor_tensor(out=qb[:, :], in0=bqk[:, :], in1=qb[:, :],
                                                op=OP.subtract)
                        for g in range(16):
                            nc.vector.tensor_scalar(
                                out=xa[g][:, :], in0=xa[g][:, :],
                                scalar1=qa[:, g:g + 1], scalar2=qb[:, g:g + 1],
                                op0=OP.mult, op1=OP.add)

                        # ---------------- P3: v natural + stats ----------------
                        with tc.tile_pool(name=f"psV{rep}", bufs=1, space="PSUM") as psV:
                            ps_vs = [None, None]
                            ps_vq = [None, None]
                            for Qv in range(2):      # v channels: d in [2048+512Qv, ...)
                                wq = merge_quarter(2048 + 512 * Qv)
                                ps_vs[Qv] = psV.tile([1, 512], F32, tag=f"vs{Qv}",
                                                     name=f"psvs{Qv}")
                                ps_vq[Qv] = psV.tile([1, 512], F32, tag=f"vq{Qv}",
                                                     name=f"psvq{Qv}")
                                for tt in range(NT):
                                    if Qv == 0 and vnat[tt] is None:
                                        vnat[tt] = vpool.tile([128, C], F32R,
                                                              tag=f"v{tt}", name=f"v{tt}")
                                    ps = psA.tile([128, 512], F32, tag="mm", name=f"vps{Qv}_{tt}")
                                    for k in range(8):
                                        nc.tensor.matmul(
                                            ps[:, :],
                                            xt[k][:, 128 * tt:128 * (tt + 1)],
                                            wq[k][:, 0:512],
                                            start=(k == 0), stop=(k == 7))
                                    nc.scalar.copy(
                                        out=vnat[tt][:, 512 * Qv:512 * (Qv + 1)], in_=ps[:, :])
                                    sq = misc.tile([128, 512], F32R, tag="sq", bufs=1,
                                                   name=f"sq{Qv}_{tt}")
                                    nc.scalar.activation(
                                        out=sq[:, :], in_=vnat[tt][:, 512 * Qv:512 * (Qv + 1)],
                                        func=ACTF.Square)
                                    nc.tensor.matmul(ps_vs[Qv][0:1, :], ones_r[:, :],
                                                     vnat[tt][:, 512 * Qv:512 * (Qv + 1)],
                                                     start=(tt == 0), stop=(tt == NT - 1))
                                    nc.tensor.matmul(ps_vq[Qv][0:1, :], ones_r[:, :],
                                                     sq[:, :],
                                                     start=(tt == 0), stop=(tt == NT - 1))
                                vst1 = misc.tile([1, 512], F32, tag="vst", bufs=2,
                                                 name=f"vst1_{Qv}")
                                nc.vector.tensor_copy(out=vst1[0:1, :], in_=ps_vs[Qv][0:1, :])
                                nc.sync.dma_start(
                                    out=vstats_in[512 * Qv:512 * (Qv + 1)], in_=vst1[0:1, :])
                                vst2 = misc.tile([1, 512], F32, tag="vst", bufs=2,
                                                 name=f"vst2_{Qv}")
                                nc.vector.tensor_copy(out=vst2[0:1, :], in_=ps_vq[Qv][0:1, :])
                                nc.sync.dma_start(
                                    out=vstats_in[C + 512 * Qv:C + 512 * (Qv + 1)], in_=vst2[0:1, :])
                            if Qv == 1:
                                if single_core or no_collective:
                                    nc.sync.dma_start(out=vstats_out[:], in_=vstats_in[:])
                                else:
                                    nc.gpsimd.collective_compute(
                                        "AllReduce", OP.add,
                                        replica_groups=[list(range(NCORES))],
                                        ins=[vstats_in[:]], outs=[vstats_out[:]])

            with tc.tile_pool(name=f"bc{rep}", bufs=1) as bcp:
                rstage = bcp.tile([128, T], F32)   # row 0 holds r, then 1/r
                # ---------------- P5: scores^T, exp, causal, row sums ----
                ae = {}
                scale = 1.0 / float(np.sqrt(C))
                with tc.tile_pool(name=f"psR{rep}", bufs=1, space="PSUM") as psR:
                    for tch in range(TQ):
                        acts = [st for st in range(NT) if 128 * st < 512 * (tch + 1)]
                        ps_r = psR.tile([1, 512], F32, tag=f"r{tch}", name=f"psr{tch}")
                        for ii, st in enumerate(acts):
                            ps = psA.tile([128, 512], F32, tag="mm", name=f"scps{tch}_{st}")
                            for j in range(8):
                                nc.tensor.matmul(
                                    ps[:, :],
                                    xa[8 + j][:, 128 * st:128 * (st + 1)],
                                    xa[j][:, 512 * tch:512 * (tch + 1)],
                                    start=(j == 0), stop=(j == 7))
                            a_t = attp.tile([128, 512], F32R, tag=f"ae{tch}_{st}",
                                            name=f"ae{tch}_{st}")
                            nc.scalar.activation(out=a_t[:, :], in_=ps[:, :],
                                                 func=ACTF.Exp, scale=scale)
                            base = 512 * tch - 128 * st
                            if base < 127:
                                nc.gpsimd.affine_select(
                                    out=a_t[:, :], in_=a_t[:, :],
                                    pattern=[[1, 512]], base=base,
                                    channel_multiplier=-1,
                                    compare_op=OP.is_ge, fill=0.0)
                            nc.tensor.matmul(ps_r[0:1, :], ones_r[:, :], a_t[:, :],
                                             start=(ii == 0), stop=(ii == len(acts) - 1))
                            ae[(tch, st)] = a_t
                        nc.vector.tensor_copy(out=rstage[0:1, 512 * tch:512 * (tch + 1)],
                                              in_=ps_r[0:1, :])
                    nc.vector.reciprocal(out=rstage[0:1, :], in_=rstage[0:1, :])
                    nc.sync.dma_start(out=rb_dram[:], in_=rstage[0:1, :])
                    nc.sync.dma_start(out=r_bc[:, :], in_=bcast_dram(rb_dram, 0, T))

            # ---------------- v scale/bias math (readback emitted post-P5) ----
            # y_final = (att_exp @ v_raw) * scale_v / r + bias_v  (v BN folded into
            # the AV drain; scale/bias are per-partition in the y^T layout)
            gv8 = misc.tile([128, 8], F32)
            nc.sync.dma_start(out=gv8[:, :], in_=gam[2048:3072].rearrange("(i p) -> p i", p=128))
            bv8 = misc.tile([128, 8], F32)
            nc.sync.dma_start(out=bv8[:, :], in_=bet[2048:3072].rearrange("(i p) -> p i", p=128))
            vs_m = misc.tile([128, 8], F32)
            nc.sync.dma_start(out=vs_m[:, :], in_=vstats_out[0:C].rearrange("(i p) -> p i", p=128))
            vs_e = misc.tile([128, 8], F32)
            nc.sync.dma_start(out=vs_e[:, :], in_=vstats_out[C:2 * C].rearrange("(i p) -> p i", p=128))
            m8 = misc.tile([128, 8], F32)
            va = misc.tile([128, 8], F32)
            vb = misc.tile([128, 8], F32)
            inv_n = 1.0 / (NCORES * T)
            nc.vector.tensor_scalar(out=vs_m[:, :], in0=vs_m[:, :],
                                    scalar1=inv_n, scalar2=None, op0=OP.mult)
            nc.vector.tensor_scalar(out=vs_e[:, :], in0=vs_e[:, :],
                                    scalar1=inv_n, scalar2=None, op0=OP.mult)
            nc.vector.tensor_tensor(out=m8[:, :], in0=vs_m[:, :], in1=vs_m[:, :], op=OP.mult)
            nc.vector.tensor_tensor(out=m8[:, :], in0=vs_e[:, :], in1=m8[:, :], op=OP.subtract)
            nc.scalar.activation(out=m8[:, :], in_=m8[:, :], func=ACTF.Sqrt,
                                 bias=eps_t[:, 0:1])
            nc.vector.reciprocal(out=m8[:, :], in_=m8[:, :])
            nc.vector.tensor_tensor(out=va[:, :], in0=m8[:, :], in1=gv8[:, :], op=OP.mult)
            nc.vector.tensor_tensor(out=vb[:, :], in0=vs_m[:, :], in1=va[:, :], op=OP.mult)
            nc.vector.tensor_tensor(out=vb[:, :], in0=bv8[:, :], in1=vb[:, :], op=OP.subtract)

        # xapool closed (frees 64KB/partition for the projection weights)
        with (
            tc.tile_pool(name=f"projp{rep}", bufs=1) as projp,
            tc.tile_pool(name=f"psP{rep}", bufs=2, space="PSUM") as psP,
        ):
            # ---------------- P6: AV + fused 1/r ----------------
            y = [None] * 8
            for tch in range(TQ):
                acts = [st for st in range(NT) if 128 * st < 512 * (tch + 1)]
                for ct in range(8):
                    ps = psA.tile([128, 512], F32, tag="mm", name=f"avps{tch}_{ct}")
                    for ii, st in enumerate(acts):
                        nc.tensor.matmul(
                            ps[:, :],
                            vnat[st][:, 128 * ct:128 * (ct + 1)],
                            ae[(tch, st)][:, :],
                            start=(ii == 0), stop=(ii == len(acts) - 1))
                    if y[ct] is None:
                        y[ct] = projp.tile([128, T], F32R, tag=f"y{ct}", name=f"y{ct}")
                    ysl = y[ct][:, 512 * tch:512 * (tch + 1)]
                    nc.vector.tensor_tensor(
                        out=ysl, in0=ps[:, :], in1=r_bc[:, 512 * tch:512 * (tch + 1)],
                        op=OP.mult)
                    nc.vector.tensor_scalar(
                        out=ysl, in0=ysl,
                        scalar1=va[:, ct:ct + 1], scalar2=vb[:, ct:ct + 1],
                        op0=OP.mult, op1=OP.add)

            with tc.tile_pool(name=f"lorap2{rep}", bufs=1) as lorap2:
                lpa_sb = lorap2.tile([R, C], F32R)
                nc.sync.dma_start(out=lpa_sb[:, :], in_=lpaT[:, :])
                lpb_sb = lorap2.tile([R, C], F32R)
                nc.sync.dma_start(out=lpb_sb[:, :], in_=lpbB[:, :])

                wp = []
                wmp = []
                for ct in range(8):
                    w1 = projp.tile([128, C], F32R, tag=f"wp{ct}", name=f"wp{ct}")
                    nc.sync.dma_start(out=w1[:, :], in_=wpT[128 * ct:128 * (ct + 1), :])
                    wp.append(w1)
                    w2 = projp.tile([128, C], F32R, tag=f"wmp{ct}", name=f"wmp{ct}")
                    nc.sync.dma_start(out=w2[:, :], in_=wpT[128 * ct:128 * (ct + 1), :])
                    wmp.append(w2)
                for et in range(8):
                    for fc in range(2):
                        ps = psA.tile([128, 512], F32, tag="mm", name=f"dpps{et}_{fc}")
                        nc.tensor.matmul(
                            ps[:, :],
                            lpb_sb[:, 128 * et:128 * (et + 1)],
                            lpa_sb[:, 512 * fc:512 * (fc + 1)],
                            start=True, stop=True)
                        nc.vector.tensor_tensor(
                            out=wmp[et][:, 512 * fc:512 * (fc + 1)],
                            in0=wmp[et][:, 512 * fc:512 * (fc + 1)],
                            in1=ps[:, :], op=OP.add)

                # ---------------- P7: double projection ----------------

                y1 = [None] * 8
                for tch in range(TQ):
                    for et in range(8):
                        ps = psP.tile([128, 512], F32, tag="pp", name=f"p1ps{tch}_{et}")
                        for ct in range(8):
                            nc.tensor.matmul(
                                ps[:, :],
                                wp[ct][:, 128 * et:128 * (et + 1)],
                                y[ct][:, 512 * tch:512 * (tch + 1)],
                                start=(ct == 0), stop=(ct == 7))
                        if y1[et] is None:
                            y1[et] = projp.tile([128, T], F32R, tag=f"y1{et}",
                                                 name=f"y1_{et}")
                        nc.scalar.copy(out=y1[et][:, 512 * tch:512 * (tch + 1)],
                                       in_=ps[:, :])
                for tch in range(TQ):
                    for ft in range(8):
                        ps = psP.tile([128, 512], F32, tag="pp", name=f"p2ps{tch}_{ft}")
                        for et in range(8):
                            nc.tensor.matmul(
                                ps[:, :],
                                wmp[et][:, 128 * ft:128 * (ft + 1)],
                                y1[et][:, 512 * tch:512 * (tch + 1)],
                                start=(et == 0), stop=(et == 7))
                        o_t = outst.tile([128, 512], F32, tag="o", name=f"o{tch}_{ft}")
                        nc.vector.tensor_copy(out=o_t[:, :], in_=ps[:, :])
                        nc.sync.dma_start(
                            out=out[128 * ft:128 * (ft + 1), 512 * tch:512 * (tch + 1)],
                            in_=o_t[:, :])


_NC_CACHE = {}


def _get_nc(T):
    if T not in _NC_CACHE:
        _NC_CACHE[T] = build(T)
    return _NC_CACHE[T]


LAST_RESULTS = None
LAST_IN_MAPS = None


def kernel(x, W_attn, W_proj, lora_attn_A, lora_attn_B, lora_proj_A, lora_proj_B,
           bn_gamma, bn_beta):
    global LAST_RESULTS, LAST_IN_MAPS
    f = np.float32
    x = np.asarray(x, f)
    B, T, C_ = x.shape
    assert C_ == C and B == NCORES

    wT = np.ascontiguousarray(np.asarray(W_attn, f).T)      # [C, 3C]
    wpT = np.ascontiguousarray(np.asarray(W_proj, f).T)     # [C, C]
    laT = np.ascontiguousarray(np.asarray(lora_attn_A, f).T)   # [R, C]
    lbB = np.ascontiguousarray(np.asarray(lora_attn_B, f))     # [R, 3C]
    lpaT = np.ascontiguousarray(np.asarray(lora_proj_A, f).T)  # [R, C]
    lpbB = np.ascontiguousarray(np.asarray(lora_proj_B, f))    # [R, C]
    gam = np.ascontiguousarray(np.asarray(bn_gamma, f))
    bet = np.ascontiguousarray(np.asarray(bn_beta, f))

    in_maps = []
    for b in range(B):
        in_maps.append({
            "xT": np.ascontiguousarray(x[b].T),
            "wT": wT, "wpT": wpT, "laT": laT, "lbB": lbB,
            "lpaT": lpaT, "lpbB": lpbB, "gam": gam, "bet": bet,
        })

    LAST_IN_MAPS = in_maps
    nc = _get_nc(T)
    res = run_bass_kernel_spmd(nc, in_maps, core_ids=list(range(NCORES)))
    LAST_RESULTS = res
    return np.stack([np.asarray(res.results[b]["out"]).T for b in range(B)]).astype(f)



# revision 28
# speedup vs baseline: 83.5239x; 83.5239x over previous
"""Trainium2 Bass kernel for nn_Attention_4_lora (B=8, T=1024, C=1024, R=64).

Strategy: data-parallel over the batch dim (1 batch per NeuronCore, 8 cores).
All activations live in transposed [channel, token] layout so that every
matmul contraction runs over the SBUF partition axis. BatchNorm statistics
are reduced across cores with two small AllReduces. Heavy matmuls run in
float32r (TF32-like, full PE throughput at N>=256); q,k activations are
kept in bf16 (same PE rate, half the SBUF/DVE traffic).

Per-core pipeline:
  P1  merge Wm_attn^T = W_attn^T + reshape(A@B)^T on device, in d-HALVES
      (1024 channels -> delta matmuls have moving dim ~342, full f32r rate).
      Weight tiles are double-buffered so the next half's HBM load overlaps
      the current half's compute; the strided delta adds alternate between
      the Vector and GpSimd engines.
  P2  xa^T[d, t] = Wm^T-slab.T @ x^T for q (half 0) and k (half 1),
      bn_stats per tile
  WEFF (between the two P2 halves) this core's 128-row shard of
      W_eff^T = Wp^T @ Wmp^T  (Wmp = Wp + lpA@lpB), via
      Z = lpB @ Wp[:, shard]  then  shard = Wp[:, shard]^T Wp^T + Z^T lpA^T
      -- all chained N=512 matmuls; AllGather the 8 shards -> every core
      holds the full [C, C] W_eff^T. Replaces the replicated double
      projection with a single projection pass.
  P3  v[t, c] (natural layout, needed as AV stationary) + ones-matmul stats
  AR  two AllReduces (qk stats after P2, v stats after P3) overlap P3/P5
  P5  scores^T[s, t] = k^T-slab.T @ q^T, exp((q.k)/32) on ScalarE,
      causal mask via affine_select (exact zeros), row-sums via ones-matmul
  P6  y^T[c, t] = v-slab.T @ att_exp^T, fused 1/r + BN-v normalize on drain
  P7  single projection: out^T = W_eff^T-slab.T @ y^T -> [C, T]

kernel() takes the full unsharded inputs, shards/uploads, runs SPMD on
cores 0-7, gathers, and transposes back to [B, T, C].
"""

import numpy as np

import concourse.bass as bass
import concourse.mybir as mybir
import concourse.tile as tile
from concourse import bacc
from concourse.bass_utils import run_bass_kernel_spmd

NCORES = 8
C = 1024
R = 64
D3 = 3 * C
EPS = 1e-5
F32 = mybir.dt.float32
F32R = mybir.dt.float32r
BF16 = mybir.dt.bfloat16
AX = mybir.AxisListType
OP = mybir.AluOpType
ACTF = mybir.ActivationFunctionType


def _erange(f, d0, d1):
    """e-range such that d = 3e + f lies in [d0, d1)."""
    el = -((-(d0 - f)) // 3)
    eh = -((-(d1 - f)) // 3)
    return el, eh


def build(T=1024, single_core=False, no_collective=False, reps=1):
    NT = T // 128          # 128-token tiles
    TQ = T // 512          # 512-token chunks
    assert T % 512 == 0

    nc = bacc.Bacc(None, target_bir_lowering=False,
                   num_devices=(1 if single_core else NCORES))

    prm = {}
    prm["xT"] = nc.declare_dram_parameter("xT", [C, T], F32R, isOutput=False)
    prm["wT"] = nc.declare_dram_parameter("wT", [C, D3], F32R, isOutput=False)
    prm["wpT"] = nc.declare_dram_parameter("wpT", [C, C], F32R, isOutput=False)
    prm["wpN"] = nc.declare_dram_parameter("wpN", [C, 128], F32R, isOutput=False)
    prm["laT"] = nc.declare_dram_parameter("laT", [R, C], F32R, isOutput=False)
    prm["lbB"] = nc.declare_dram_parameter("lbB", [R, D3], F32R, isOutput=False)
    prm["lpaT"] = nc.declare_dram_parameter("lpaT", [R, C], F32R, isOutput=False)
    prm["lpbN"] = nc.declare_dram_parameter("lpbN", [C, R], F32R, isOutput=False)
    prm["gam"] = nc.declare_dram_parameter("gam", [D3], F32, isOutput=False)
    prm["bet"] = nc.declare_dram_parameter("bet", [D3], F32, isOutput=False)
    prm["out"] = nc.declare_dram_parameter("out", [C, T], F32, isOutput=True)

    with tile.TileContext(nc) as tc:
        for rep in range(reps):
            _emit(nc, tc, prm, T, rep, single_core, no_collective)

    nc.compile()
    return nc


def _emit(nc, tc, prm, T, rep, single_core, no_collective):
    NT = T // 128
    TQ = T // 512
    xT, wT, wpT, wpN = prm["xT"], prm["wT"], prm["wpT"], prm["wpN"]
    laT, lbB = prm["laT"], prm["lbB"]
    lpaT, lpbN, gam, bet, out = prm["lpaT"], prm["lpbN"], prm["gam"], prm["bet"], prm["out"]

    stats_in = nc.dram_tensor(f"stats_in_{rep}", [4096], F32)
    stats_out = nc.dram_tensor(f"stats_out_{rep}", [4096], F32)
    vstats_in = nc.dram_tensor(f"vstats_in_{rep}", [2 * C], F32)
    vstats_out = nc.dram_tensor(f"vstats_out_{rep}", [2 * C], F32)
    rb_dram = nc.dram_tensor(f"rb_{rep}", [T], F32)
    weff_in = nc.dram_tensor(f"weff_in_{rep}", [128 * C], F32R)
    bias_dram = nc.dram_tensor(f"bias_{rep}", [C], F32)
    weff_out = nc.dram_tensor(f"weff_out_{rep}", [C * C], F32R,
                              addr_space="Shared")

    def bcast_dram(param, offset, n):
        return bass.AP(tensor=param[:].tensor, offset=offset, ap=[[0, 128], [1, n]])

    def all_reduce(ins, outs):
        if single_core or no_collective:
            nc.sync.dma_start(out=outs, in_=ins)
        else:
            nc.gpsimd.collective_compute(
                "AllReduce", OP.add,
                replica_groups=[list(range(NCORES))],
                ins=[ins], outs=[outs])

    with (
        tc.tile_pool(name=f"misc{rep}", bufs=1) as misc,
        tc.tile_pool(name=f"outst{rep}", bufs=2) as outst,
        tc.tile_pool(name=f"vpool{rep}", bufs=1) as vpool,
        tc.tile_pool(name=f"attp{rep}", bufs=1) as attp,
        tc.tile_pool(name=f"psA{rep}", bufs=4, space="PSUM") as psA,
    ):
        # ---------------- constants / small loads ----------------
        ones_f = misc.tile([128, 1], F32)
        nc.vector.memset(ones_f[:, :], 1.0)
        ones_r = misc.tile([128, 1], F32R)
        nc.vector.tensor_copy(out=ones_r[:, :], in_=ones_f[:, :])
        ones_b = misc.tile([128, 1], BF16)
        nc.vector.tensor_copy(out=ones_b[:, :], in_=ones_f[:, :])
        eps_t = misc.tile([128, 1], F32)
        nc.vector.memset(eps_t[:, :], EPS)

        qk_mv = misc.tile([128, 16, 2], F32)
        m16 = misc.tile([128, 16], F32)
        qa = misc.tile([128, 16], F32)
        qb = misc.tile([128, 16], F32)

        xa = [None] * 16
        vnat = [None] * NT
        weff = [None] * 8

        with tc.tile_pool(name=f"xapool{rep}", bufs=1) as xapool:
            with tc.tile_pool(name=f"lorap{rep}", bufs=1) as lorap:
                la_sb = lorap.tile([R, C], F32R)
                for _c in range(2):
                    nc.sync.dma_start(out=la_sb[:, 512 * _c:512 * (_c + 1)],
                                      in_=laT[:, 512 * _c:512 * (_c + 1)])
                lb_sb = lorap.tile([R, D3], F32R)
                for _c in range(6):
                    nc.sync.dma_start(out=lb_sb[:, 512 * _c:512 * (_c + 1)],
                                      in_=lbB[:, 512 * _c:512 * (_c + 1)])

                with tc.tile_pool(name=f"xtpool{rep}", bufs=1) as xtpool:
                    with tc.tile_pool(name=f"wb{rep}", bufs=1) as wbp:
                        # -------- P1 merge (d-halves, double-buffered) ------
                        def merge_half(d0):
                            """Merged Wm^T[:, d0:d0+1024] as 8 c-tiles [128, 1032]."""
                            wq = []
                            for ct in range(8):
                                w_t = wbp.tile([128, 1032], F32R, tag=f"wb{ct}",
                                               bufs=(2 if ct < 4 else 1),
                                               name=f"wh{d0}_{ct}")
                                nc.sync.dma_start(
                                    out=w_t[:, 0:1024],
                                    in_=wT[128 * ct:128 * (ct + 1), d0:d0 + 1024])
                                wq.append(w_t)
                            for ct in range(8):
                                w_t = wq[ct]
                                view3 = w_t[:, :].rearrange("p (u three) -> p u three", three=3)
                                for f in range(3):
                                    el, eh = _erange(f, d0, d0 + 1024)
                                    cnt = eh - el
                                    c0 = 3 * el + f - d0
                                    # f32r matmul needs an even moving free dim
                                    cnt_mm = cnt + (cnt % 2)
                                    es, off = el, 0
                                    if es + cnt_mm > C:
                                        es, off = el - 1, 1
                                    ps = psA.tile([128, 512], F32, tag="mm", name=f"dps{d0}_{ct}_{f}")
                                    nc.tensor.matmul(
                                        ps[:, 0:cnt_mm],
                                        lb_sb[:, 1024 * f + 128 * ct:1024 * f + 128 * (ct + 1)],
                                        la_sb[:, es:es + cnt_mm],
                                        start=True, stop=True)
                                    if (ct + f) % 2 == 0:
                                        nc.vector.tensor_tensor(
                                            out=view3[:, 0:cnt, c0],
                                            in0=view3[:, 0:cnt, c0],
                                            in1=ps[:, off:off + cnt], op=OP.add)
                                    else:
                                        # GpSimd cannot read PSUM: stage via the
                                        # (idle) Scalar engine, add on GpSimd
                                        stg = misc.tile([128, 344], F32R,
                                                        tag="mgst", bufs=2,
                                                        name=f"mgst{d0}_{ct}_{f}")
                                        nc.scalar.copy(out=stg[:, 0:cnt_mm],
                                                       in_=ps[:, 0:cnt_mm])
                                        nc.gpsimd.tensor_tensor(
                                            out=view3[:, 0:cnt, c0],
                                            in0=view3[:, 0:cnt, c0],
                                            in1=stg[:, off:off + cnt], op=OP.add)
                            return wq

                        wq0 = merge_half(0)
                        xt = []
                        for k in range(8):
                            x_t = xtpool.tile([128, T], F32R, tag=f"xt{k}", name=f"xt{k}")
                            nc.sync.dma_start(out=x_t[:, :], in_=xT[128 * k:128 * (k + 1), :])
                            xt.append(x_t)

                        def p2_half(H, wq):
                            for il in range(8):
                                g = 8 * H + il
                                xa_g = xapool.tile([128, T], BF16, tag=f"xa{g}",
                                                   name=f"xa{g}")
                                # two interleaved chains (one per 512-token
                                # chunk) hide the per-chain ldweights bubbles
                                pss = [psA.tile([128, 512], F32, tag="mm",
                                                name=f"xaps{g}_{tch}")
                                       for tch in range(TQ)]
                                for k in range(8):
                                    for tch in range(TQ):
                                        nc.tensor.matmul(
                                            pss[tch][:, :],
                                            wq[k][:, 128 * il:128 * (il + 1)],
                                            xt[k][:, 512 * tch:512 * (tch + 1)],
                                            start=(k == 0), stop=(k == 7))
                                for tch in range(TQ):
                                    nc.scalar.copy(out=xa_g[:, 512 * tch:512 * (tch + 1)],
                                                   in_=pss[tch][:, :])
                                bnstat = misc.tile([128, TQ, 6], F32, tag="bnstat",
                                                   bufs=2, name=f"bnstat{g}")
                                for j in range(TQ):
                                    nc.vector.bn_stats(out=bnstat[:, j, :],
                                                       in_=xa_g[:, 512 * j:512 * (j + 1)])
                                nc.vector.bn_aggr(out=qk_mv[:, g, :], in_=bnstat[:, :, :])
                                xa[g] = xa_g

                        p2_half(0, wq0)          # q channels d in [0, 1024)

                        # -------- WEFF: this core's 128-col shard of
                        # W_eff^T = Wp^T @ Wmp^T with Wmp = Wp + lpA@lpB:
                        #   Z = lpB @ Wp[:, shard]          ([R, 128])
                        #   shard^T = Wp[:,shard]^T Wp^T + Z^T lpA^T
                        # then AllGather -> full [C, C] W_eff^T.
                        with tc.tile_pool(name=f"wefc{rep}", bufs=1) as wefc:
                            lpa_sb = wefc.tile([R, C], F32R)
                            nc.sync.dma_start(out=lpa_sb[:, :], in_=lpaT[:, :])
                            lpbn = wefc.tile([128, 8 * R], F32R)
                            for et in range(8):
                                nc.sync.dma_start(
                                    out=lpbn[:, R * et:R * (et + 1)],
                                    in_=lpbN[128 * et:128 * (et + 1), :])
                            wpn = wefc.tile([128, C], F32R)
                            for et in range(8):
                                nc.sync.dma_start(
                                    out=wpn[:, 128 * et:128 * (et + 1)],
                                    in_=wpN[128 * et:128 * (et + 1), :])
                            z_sb = wefc.tile([R, 128], F32R)
                            ps = psA.tile([128, 512], F32, tag="mm", name="zps")
                            for et in range(8):
                                nc.tensor.matmul(
                                    ps[0:R, 0:128],
                                    lpbn[:, R * et:R * (et + 1)],
                                    wpn[:, 128 * et:128 * (et + 1)],
                                    start=(et == 0), stop=(et == 7))
                            nc.scalar.copy(out=z_sb[:, :], in_=ps[0:R, 0:128])
                            for fc in range(2):
                                ps = psA.tile([128, 512], F32, tag="mm",
                                              name=f"weffps{fc}")
                                for et in range(8):
                                    w2 = wefc.tile([128, 512], F32R, tag=f"wpt{et}",
                                                   bufs=1, name=f"wpt{fc}_{et}")
                                    nc.sync.dma_start(
                                        out=w2[:, :],
                                        in_=wpT[128 * et:128 * (et + 1),
                                                512 * fc:512 * (fc + 1)])
                                    nc.tensor.matmul(
                                        ps[:, :], wpn[:, 128 * et:128 * (et + 1)],
                                        w2[:, :],
                                        start=(et == 0), stop=False)
                                nc.tensor.matmul(
                                    ps[:, :], z_sb[:, :],
                                    lpa_sb[:, 512 * fc:512 * (fc + 1)],
                                    start=False, stop=True)
                                wst = wefc.tile([128, 512], F32R, tag="wst", bufs=1,
                                                name=f"weffst{fc}")
                                nc.vector.tensor_copy(out=wst[:, :], in_=ps[:, :])
                                nc.sync.dma_start(
                                    out=bass.AP(tensor=weff_in[:].tensor,
                                                offset=512 * fc,
                                                ap=[[C, 128], [1, 512]]),
                                    in_=wst[:, :])
                        if single_core or no_collective:
                            # local fallback: replicate shard into all 8 slots
                            for ct in range(8):
                                nc.sync.dma_start(
                                    out=weff_out[128 * C * ct:128 * C * (ct + 1)],
                                    in_=weff_in[:])
                        else:
                            nc.gpsimd.collective_compute(
                                "AllGather", OP.bypass,
                                replica_groups=[list(range(NCORES))],
                                ins=[weff_in[:]], outs=[weff_out[:]])

                        wq1 = merge_half(1024)
                        p2_half(1, wq1)          # k channels d in [1024, 2048)

                        # qk stats -> (mean, E[x^2]) packed, DMA to stats_in
                        nc.vector.tensor_tensor(out=m16[:, :], in0=qk_mv[:, :, 0],
                                                in1=qk_mv[:, :, 0], op=OP.mult)
                        nc.vector.tensor_tensor(out=qk_mv[:, :, 1], in0=qk_mv[:, :, 1],
                                                in1=m16[:, :], op=OP.add)
                        nc.sync.dma_start(
                            out=stats_in[0:4096].rearrange("(p i s) -> p i s", p=128, s=2),
                            in_=qk_mv[:, :, :])
                        all_reduce(stats_in[:], stats_out[:])

                        # ---------------- P3: v natural + stats ----------------
                        with tc.tile_pool(name=f"psV{rep}", bufs=1, space="PSUM") as psV:
                            wqv = merge_half(2048)

                            # qk-stats readback + normalize: all Vector-engine
                            # (rsqrt via DVE pow) so nothing fences the Scalar
                            # P3 drain queue; runs as soon as the AllReduce
                            # lands, overlapping the P3 matmuls.
                            gqk = misc.tile([128, 16], F32)
                            nc.sync.dma_start(out=gqk[:, :],
                                              in_=gam[0:2048].rearrange("(i p) -> p i", p=128))
                            bqk = misc.tile([128, 16], F32)
                            nc.sync.dma_start(out=bqk[:, :],
                                              in_=bet[0:2048].rearrange("(i p) -> p i", p=128))
                            ar_qk = misc.tile([128, 16, 2], F32)
                            nc.sync.dma_start(
                                out=ar_qk[:, :, :],
                                in_=stats_out[0:4096].rearrange("(p i s) -> p i s", p=128, s=2))
                            # q,k: a = gamma*rstd, b = beta - mean*a
                            nc.vector.tensor_scalar(out=ar_qk[:, :, 0], in0=ar_qk[:, :, 0],
                                                    scalar1=1.0 / NCORES, scalar2=None, op0=OP.mult)
                            nc.vector.tensor_scalar(out=ar_qk[:, :, 1], in0=ar_qk[:, :, 1],
                                                    scalar1=1.0 / NCORES, scalar2=None, op0=OP.mult)
                            nc.vector.tensor_tensor(out=m16[:, :], in0=ar_qk[:, :, 0],
                                                    in1=ar_qk[:, :, 0], op=OP.mult)
                            nc.vector.tensor_tensor(out=m16[:, :], in0=ar_qk[:, :, 1],
                                                    in1=m16[:, :], op=OP.subtract)

                            def finish_norm():
                                # emitted mid-P3 so the Scalar queue reaches the
                                # Sqrt only after the AllReduce has landed
                                nc.scalar.activation(out=m16[:, :], in_=m16[:, :],
                                                     func=ACTF.Sqrt, bias=eps_t[:, 0:1])
                                nc.vector.reciprocal(out=m16[:, :], in_=m16[:, :])
                                nc.vector.tensor_tensor(out=qa[:, :], in0=m16[:, :],
                                                        in1=gqk[:, :], op=OP.mult)
                                nc.vector.tensor_tensor(out=qb[:, :], in0=ar_qk[:, :, 0],
                                                        in1=qa[:, :], op=OP.mult)
                                nc.vector.tensor_tensor(out=qb[:, :], in0=bqk[:, :],
                                                        in1=qb[:, :], op=OP.subtract)
                                for g in range(16):
                                    nc.vector.tensor_scalar(
                                        out=xa[g][:, :], in0=xa[g][:, :],
                                        scalar1=qa[:, g:g + 1], scalar2=qb[:, g:g + 1],
                                        op0=OP.mult, op1=OP.add)
                            ps_vq = [None, None]
                            for hc in range(2):      # v cols [512hc, 512hc+512)
                                ps_vq[hc] = psV.tile([1, 512], F32, tag=f"vq{hc}",
                                                     name=f"psvq{hc}")
                            pend = []

                            def flush_stat():
                                hc_, tt_, sq_ = pend.pop(0)
                                nc.tensor.matmul(ps_vq[hc_][0:1, :], ones_r[:, :],
                                                 sq_[:, :],
                                                 start=(tt_ == 0), stop=(tt_ == NT - 1))
                                if tt_ == NT - 1:
                                    vst2 = misc.tile([1, 512], F32, tag="vst", bufs=2,
                                                     name=f"vst2_{hc_}")
                                    nc.scalar.copy(out=vst2[0:1, :], in_=ps_vq[hc_][0:1, :])
                                    nc.sync.dma_start(
                                        out=vstats_in[C + 512 * hc_:C + 512 * (hc_ + 1)],
                                        in_=vst2[0:1, :])

                            for tt in range(NT):
                                vnat[tt] = vpool.tile([128, C], BF16,
                                                      tag=f"v{tt}", name=f"v{tt}")
                                pss = [psA.tile([128, 512], F32, tag="mm",
                                                name=f"vps{hc}_{tt}")
                                       for hc in range(2)]
                                for k in range(8):
                                    for hc in range(2):
                                        nc.tensor.matmul(
                                            pss[hc][:, :],
                                            xt[k][:, 128 * tt:128 * (tt + 1)],
                                            wqv[k][:, 512 * hc:512 * (hc + 1)],
                                            start=(k == 0), stop=(k == 7))
                                for hc in range(2):
                                    nc.scalar.copy(
                                        out=vnat[tt][:, 512 * hc:512 * (hc + 1)],
                                        in_=pss[hc][:, :])
                                    sq = misc.tile([128, 512], F32R, tag="sq", bufs=3,
                                                   name=f"sq{hc}_{tt}")
                                    nc.scalar.activation(
                                        out=sq[:, :], in_=pss[hc][:, :], func=ACTF.Square)
                                    pend.append((hc, tt, sq))
                                while len(pend) >= 3:
                                    flush_stat()
                                if tt == 2:
                                    finish_norm()
                            while pend:
                                flush_stat()

            # projp opens BEFORE P5 so the W_eff readback DMAs sit ahead of
            # the AllReduce-gated stat readbacks in the serial Sync queue.
            with tc.tile_pool(name=f"projp{rep}", bufs=1) as projp:
                for ct in range(8):
                    weff[ct] = projp.tile([128, C], F32R, tag=f"wf{ct}", name=f"wf{ct}")
                    nc.sync.dma_start(
                        out=weff[ct][:, :],
                        in_=weff_out[128 * C * ct:128 * C * (ct + 1)].rearrange(
                            "(p i) -> p i", p=128))
                r_bc = projp.tile([128, T], F32)   # broadcast of 1/r per chunk
                with tc.tile_pool(name=f"bc{rep}", bufs=1) as bcp:
                    rstage = bcp.tile([128, T], F32)   # row 0: r -> 1/r
                    # ------------ P5: scores^T, exp, causal, row sums ----
                    ae = {}
                    scale = 1.0 / float(np.sqrt(C))
                    with tc.tile_pool(name=f"psR{rep}", bufs=1, space="PSUM") as psR:
                        items = []
                        for tch in range(TQ):
                            acts = [st for st in range(NT) if 128 * st < 512 * (tch + 1)]
                            for ii, st in enumerate(acts):
                                items.append((tch, st, ii, len(acts)))
                        ps_rs = {tch: psR.tile([1, 512], F32, tag=f"r{tch}",
                                               name=f"psr{tch}") for tch in range(TQ)}

                        def vs_sums():
                            # deferred v column sums: two 8-long chains emitted
                            # after the first P5 pair so they don't delay P5
                            for hc_ in range(2):
                                ps_vs = psR.tile([1, 512], F32, tag=f"vs{hc_}",
                                                 name=f"psvs{hc_}")
                                for tt_ in range(NT):
                                    nc.tensor.matmul(
                                        ps_vs[0:1, :], ones_b[:, :],
                                        vnat[tt_][:, 512 * hc_:512 * (hc_ + 1)],
                                        start=(tt_ == 0), stop=(tt_ == NT - 1))
                                vst1 = misc.tile([1, 512], F32, tag="vst", bufs=2,
                                                 name=f"vst1_{hc_}")
                                nc.scalar.copy(out=vst1[0:1, :], in_=ps_vs[0:1, :])
                                nc.sync.dma_start(
                                    out=vstats_in[512 * hc_:512 * (hc_ + 1)],
                                    in_=vst1[0:1, :])

                        for p0 in range(0, len(items), 2):
                            if p0 == 2:
                                vs_sums()
                                all_reduce(vstats_in[:], vstats_out[:])
                            pair = items[p0:p0 + 2]
                            pss = []
                            for (tch, st, ii, na) in pair:
                                pss.append(psA.tile([128, 512], F32, tag="mm",
                                                    name=f"scps{tch}_{st}"))
                            for j in range(8):
                                for pi, (tch, st, ii, na) in enumerate(pair):
                                    nc.tensor.matmul(
                                        pss[pi][:, :],
                                        xa[8 + j][:, 128 * st:128 * (st + 1)],
                                        xa[j][:, 512 * tch:512 * (tch + 1)],
                                        start=(j == 0), stop=(j == 7))
                            for pi, (tch, st, ii, na) in enumerate(pair):
                                a_t = attp.tile([128, 512], BF16, tag=f"ae{tch}_{st}",
                                                name=f"ae{tch}_{st}")
                                nc.scalar.activation(out=a_t[:, :], in_=pss[pi][:, :],
                                                     func=ACTF.Exp, scale=scale)
                                base = 512 * tch - 128 * st
                                if base < 127:
                                    nc.gpsimd.affine_select(
                                        out=a_t[:, :], in_=a_t[:, :],
                                        pattern=[[1, 512]], base=base,
                                        channel_multiplier=-1,
                                        compare_op=OP.is_ge, fill=0.0)
                                nc.tensor.matmul(ps_rs[tch][0:1, :], ones_b[:, :], a_t[:, :],
                                                 start=(ii == 0), stop=(ii == na - 1))
                                ae[(tch, st)] = a_t
                                if ii == na - 1:
                                    nc.scalar.copy(
                                        out=rstage[0:1, 512 * tch:512 * (tch + 1)],
                                        in_=ps_rs[tch][0:1, :])
                                    nc.vector.reciprocal(
                                        out=rstage[0:1, 512 * tch:512 * (tch + 1)],
                                        in_=rstage[0:1, 512 * tch:512 * (tch + 1)])
                                    nc.sync.dma_start(
                                        out=rb_dram[512 * tch:512 * (tch + 1)],
                                        in_=rstage[0:1, 512 * tch:512 * (tch + 1)])
                                    nc.sync.dma_start(
                                        out=r_bc[:, 512 * tch:512 * (tch + 1)],
                                        in_=bcast_dram(rb_dram, 512 * tch, 512))

                # -------- v scale/bias math (readback emitted post-P5) ----
                gv8 = misc.tile([128, 8], F32)
                nc.sync.dma_start(out=gv8[:, :], in_=gam[2048:3072].rearrange("(i p) -> p i", p=128))
                bv8 = misc.tile([128, 8], F32)
                nc.sync.dma_start(out=bv8[:, :], in_=bet[2048:3072].rearrange("(i p) -> p i", p=128))
                vs_m = misc.tile([128, 8], F32)
                nc.sync.dma_start(out=vs_m[:, :], in_=vstats_out[0:C].rearrange("(i p) -> p i", p=128))
                vs_e = misc.tile([128, 8], F32)
                nc.sync.dma_start(out=vs_e[:, :], in_=vstats_out[C:2 * C].rearrange("(i p) -> p i", p=128))
                m8 = misc.tile([128, 8], F32)
                va = misc.tile([128, 8], F32)
                vb = misc.tile([128, 8], F32)
                vbva = misc.tile([128, 8], F32)
                vbva_r = misc.tile([128, 8], F32R)
                inv_n = 1.0 / ((1 if single_core else NCORES) * T)

                def vavb_math():
                    nc.vector.tensor_scalar(out=vs_m[:, :], in0=vs_m[:, :],
                                            scalar1=inv_n, scalar2=None, op0=OP.mult)
                    nc.vector.tensor_scalar(out=vs_e[:, :], in0=vs_e[:, :],
                                            scalar1=inv_n, scalar2=None, op0=OP.mult)
                    nc.vector.tensor_tensor(out=m8[:, :], in0=vs_m[:, :], in1=vs_m[:, :], op=OP.mult)
                    nc.vector.tensor_tensor(out=m8[:, :], in0=vs_e[:, :], in1=m8[:, :], op=OP.subtract)
                    nc.scalar.activation(out=m8[:, :], in_=m8[:, :], func=ACTF.Sqrt,
                                         bias=eps_t[:, 0:1])
                    nc.vector.reciprocal(out=m8[:, :], in_=m8[:, :])
                    nc.vector.tensor_tensor(out=va[:, :], in0=m8[:, :], in1=gv8[:, :], op=OP.mult)
                    nc.vector.tensor_tensor(out=vb[:, :], in0=vs_m[:, :], in1=va[:, :], op=OP.mult)
                    nc.vector.tensor_tensor(out=vb[:, :], in0=bv8[:, :], in1=vb[:, :], op=OP.subtract)
                    nc.vector.reciprocal(out=vbva[:, :], in_=va[:, :])
                    nc.vector.tensor_tensor(out=vbva[:, :], in0=vb[:, :], in1=vbva[:, :], op=OP.mult)
                    nc.vector.tensor_copy(out=vbva_r[:, :], in_=vbva[:, :])

                # ---------------- P6: AV + fused 1/r ----------------
                # BN-v fold: W_eff' = W_eff * va so the P6 drain is a single
                # 1/r multiply; vb flows through the projection as
                # bias_out[f] = sum_c W_eff'[f,c] * (vb/va)[c], added on the
                # P7 drain. va/vb math + weff scaling are emitted between the
                # P6 drain batches so nothing fences the drain queue.
                y = [None] * 8
                for tch in range(TQ):
                    acts = [st for st in range(NT) if 128 * st < 512 * (tch + 1)]
                    for c0 in range(0, 8, 2):
                        pss = [psA.tile([128, 512], F32, tag="mm",
                                        name=f"avps{tch}_{c0 + pi}") for pi in range(2)]
                        for ii, st in enumerate(acts):
                            for pi in range(2):
                                nc.tensor.matmul(
                                    pss[pi][:, :],
                                    vnat[st][:, 128 * (c0 + pi):128 * (c0 + pi + 1)],
                                    ae[(tch, st)][:, :],
                                    start=(ii == 0), stop=(ii == len(acts) - 1))
                        for pi in range(2):
                            ct = c0 + pi
                            if y[ct] is None:
                                y[ct] = projp.tile([128, T], F32R, tag=f"y{ct}",
                                                   name=f"y{ct}")
                            nc.vector.tensor_tensor(
                                out=y[ct][:, 512 * tch:512 * (tch + 1)], in0=pss[pi][:, :],
                                in1=r_bc[:, 512 * tch:512 * (tch + 1)], op=OP.mult)
                    if tch == 0:
                        vavb_math()
                    if tch == 1:
                        for ct in range(8):
                            nc.vector.tensor_scalar(
                                out=weff[ct][:, :], in0=weff[ct][:, :],
                                scalar1=va[:, ct:ct + 1], scalar2=None, op0=OP.mult)
                        # bias_out = W_eff' @ (vb/va)
                        psb = [psA.tile([128, 512], F32, tag="mm", name=f"biasps{h}")
                               for h in range(2)]
                        for ct in range(8):
                            for h in range(2):
                                nc.tensor.matmul(
                                    psb[h][0:1, :], vbva_r[:, ct:ct + 1],
                                    weff[ct][:, 512 * h:512 * (h + 1)],
                                    start=(ct == 0), stop=(ct == 7))
                        bst = misc.tile([1, 1024], F32, tag="bst", bufs=1, name="bst")
                        nc.scalar.copy(out=bst[0:1, 0:512], in_=psb[0][0:1, :])
                        nc.scalar.copy(out=bst[0:1, 512:1024], in_=psb[1][0:1, :])
                        nc.sync.dma_start(out=bias_dram[0:C], in_=bst[0:1, :])
                        bias8 = misc.tile([128, 8], F32, tag="bias8", bufs=1,
                                          name="bias8")
                        nc.sync.dma_start(
                            out=bias8[:, :],
                            in_=bias_dram[0:C].rearrange("(i p) -> p i", p=128))

                # ------------ P7: single projection via W_eff ----------------
                with tc.tile_pool(name=f"psP{rep}", bufs=2, space="PSUM") as psP:
                  for tch in range(TQ):
                    for f0 in range(0, 8, 2):
                        pss = [psP.tile([128, 512], F32, tag=f"pp{pi}", bufs=2,
                                        name=f"p2ps{tch}_{f0 + pi}") for pi in range(2)]
                        for ct in range(8):
                            for pi in range(2):
                                nc.tensor.matmul(
                                    pss[pi][:, :],
                                    weff[ct][:, 128 * (f0 + pi):128 * (f0 + pi + 1)],
                                    y[ct][:, 512 * tch:512 * (tch + 1)],
                                    start=(ct == 0), stop=(ct == 7))
                        for pi in range(2):
                            ft = f0 + pi
                            o_t = outst.tile([128, 512], F32, tag="o", name=f"o{tch}_{ft}")
                            nc.vector.tensor_scalar(
                                out=o_t[:, :], in0=pss[pi][:, :],
                                scalar1=bias8[:, ft:ft + 1], scalar2=None, op0=OP.add)
                            nc.sync.dma_start(
                                out=out[128 * ft:128 * (ft + 1), 512 * tch:512 * (tch + 1)],
                                in_=o_t[:, :])


_NC_CACHE = {}


def _get_nc(T):
    if T not in _NC_CACHE:
        _NC_CACHE[T] = build(T)
    return _NC_CACHE[T]


LAST_RESULTS = None
LAST_IN_MAPS = None


def make_in_maps(inputs):
    f = np.float32
    x = np.asarray(inputs["x"], f)
    B = x.shape[0]
    wT = np.ascontiguousarray(np.asarray(inputs["W_attn"], f).T)      # [C, 3C]
    wp = np.asarray(inputs["W_proj"], f)
    wpT = np.ascontiguousarray(wp.T)                                  # [C, C]
    laT = np.ascontiguousarray(np.asarray(inputs["lora_attn_A"], f).T)   # [R, C]
    lbB = np.ascontiguousarray(np.asarray(inputs["lora_attn_B"], f))     # [R, 3C]
    lpaT = np.ascontiguousarray(np.asarray(inputs["lora_proj_A"], f).T)  # [R, C]
    lpbN = np.ascontiguousarray(np.asarray(inputs["lora_proj_B"], f).T)  # [C, R]
    gam = np.ascontiguousarray(np.asarray(inputs["bn_gamma"], f))
    bet = np.ascontiguousarray(np.asarray(inputs["bn_beta"], f))
    in_maps = []
    for b in range(B):
        in_maps.append({
            "xT": np.ascontiguousarray(x[b].T),
            "wT": wT, "wpT": wpT,
            "wpN": np.ascontiguousarray(wp[:, 128 * b:128 * (b + 1)]),
            "laT": laT, "lbB": lbB,
            "lpaT": lpaT, "lpbN": lpbN, "gam": gam, "bet": bet,
        })
    return in_maps


def kernel(x, W_attn, W_proj, lora_attn_A, lora_attn_B, lora_proj_A, lora_proj_B,
           bn_gamma, bn_beta):
    global LAST_RESULTS, LAST_IN_MAPS
    f = np.float32
    x = np.asarray(x, f)
    B, T, C_ = x.shape
    assert C_ == C and B == NCORES

    in_maps = make_in_maps({
        "x": x, "W_attn": W_attn, "W_proj": W_proj,
        "lora_attn_A": lora_attn_A, "lora_attn_B": lora_attn_B,
        "lora_proj_A": lora_proj_A, "lora_proj_B": lora_proj_B,
        "bn_gamma": bn_gamma, "bn_beta": bn_beta})

    LAST_IN_MAPS = in_maps
    nc = _get_nc(T)
    res = run_bass_kernel_spmd(nc, in_maps, core_ids=list(range(NCORES)))
    LAST_RESULTS = res
    return np.stack([np.asarray(res.results[b]["out"]).T for b in range(B)]).astype(f)
